# revision 36
# baseline (speedup 1.0000x reference)
"""ClassWeightedModalDownSampler Trainium2 kernel, v3.

Host packs each pixel's class c into two field-bytes interleaved in one
uint16 tensor cab [128, (hf 2, rh 2, src 2, r4 4, wch 8, pp 32)]:
  src=0 (ca): groups 0-2 as 2-bit fields t=c%3+1 at bits 2g..2g+1
  src=1 (cb): groups 4-6 at bits 2(g-4)..; group 3 at bits 6-7
(u16 packs two adjacent patch rows, lo/hi byte.)

Device: 4 uniform DVE tensor_scalar passes per (hf, rh) chunk over the
contiguous [ca-block | cb-block] columns:
  pass k<3: (x << (5-2k)) & 0x6060 -> [plane k | plane k+4]
  pass 3:   (x >> 1)      & 0x6060 -> [zeros   | plane 3  ]
Each output [128, 2048] u16 is bitcast to fp8e5 ({0,32,64,96} bytes =
{0, 2^-7, 2, 2^9}) and consumed by DoubleRow matmuls (2 k-tiles/pair)
with a block one-hot lhsT mapping partition p, pair k, tile t to PSUM
slot m = 8*(p//8) + group: S[8q+g, n] = n0*2^-7 + n1*2 + n2*512 exactly.

Decode per hf bank (all int16, A=32 encode E=32*w*n - c):
  t1 = trunc(S/2) (ACT) ; t9 = t1>>8 (DVE 4x) ; f1 = t1&255 (DVE 4x)
  f0 = S - 2*t1 (Pool STT, fp16) ; e_d = sc_d*f_d + bi_d (ACT/DVE-TS)
  m = max(e0,e1,e2) (DVE/Pool) ; PE-transpose m -> slots in free dim ;
  R = reduce_max over 8-slot groups (DVE) ; c* = 32*((R+20)>>5) - R.
Output [128, (hf, j, q)] int16, unscrambled on host. Exact for integer
class_weights with 32*64*max(w) < 32768 (reference w in {1,10}).
"""

import numpy as np
import ml_dtypes

import concourse.bass as bass
import concourse.mybir as mybir
import concourse.tile as tile
from concourse import bacc
from concourse.bass_utils import run_bass_kernel_spmd

NCORES = 8
B, H, W = 4, 1024, 2048
DSF = 8
NCLS = 20
GH, GW = H // DSF, W // DSF
ROWS = (B * H) // NCORES     # 512 label rows per core
PROWS = ROWS // DSF          # 64 patch rows per core
P = 128

_DT = mybir.dt
_ALU = mybir.AluOpType
_ACTF = mybir.ActivationFunctionType

TRACE = False
LAST_RESULTS = None

# encode offset: E = 32*w*n - c + 2048 keeps every value in [1024, 31744)
# whose int16 bits are a NORMAL fp16 pattern (exact through PE transpose)
EOFF = 2048.0
NEGB = 1024.0  # pad slots: below any real slot's E


def _luts():
    """Per-class field bytes: (ca, cb)."""
    la = np.zeros(256, dtype=np.uint8)
    lb = np.zeros(256, dtype=np.uint8)
    for c in range(NCLS):
        g, d = divmod(c, 3)
        t = d + 1
        if g < 3:
            la[c] = t << (2 * g)
        elif g == 3:
            lb[c] = t << 6
        else:
            lb[c] = t << (2 * (g - 4))
    return la, lb


def _aux_arrays(class_weights: np.ndarray):
    w = np.asarray(class_weights, dtype=np.float32)
    assert w.shape[0] == NCLS
    # lhsT [p, pair 4, t 2, m 128] one-hot: m = 8*(p//8) + group(pair, t)
    # group(k,0)=k (k<3), group(3,0)=None (zero half), group(k,1)=k+4 (k<3),
    # group(3,1)=3.
    lhst = np.zeros((P, 4, 2, P), dtype=np.float32)
    for p in range(P):
        mb = 8 * (p // 8)
        for k in range(4):
            for t in range(2):
                if k < 3:
                    g = k if t == 0 else k + 4
                elif t == 1:
                    g = 3
                else:
                    continue
                lhst[p, k, t, mb + g] = 1.0
    lhst_e5 = lhst.reshape(P, 4 * 2 * P).astype(ml_dtypes.float8_e5m2)

    # encode scale/bias [128, 3] (col d); partition m -> g = m % 8
    sc = np.zeros((P, 3), dtype=np.float32)
    bi = np.zeros((P, 3), dtype=np.float32)
    for m in range(P):
        g = m % 8
        for d in range(3):
            c = 3 * g + d
            if g == 7 or c >= NCLS:
                sc[m, d] = 0.0
                bi[m, d] = NEGB
            else:
                sc[m, d] = 32.0 * w[c] * (128.0 if d == 0 else 1.0)
                bi[m, d] = EOFF - float(c)
    return lhst_e5, sc, bi


def _build():
    nc = bacc.Bacc(
        "TRN2",
        target_bir_lowering=False,
        debug=False,
        num_devices=NCORES,
    )
    cab_d = nc.dram_tensor("cab", [P, 8192], _DT.uint16, kind="ExternalInput").ap()
    lhst_d = nc.dram_tensor("lhst", [P, 4 * 2 * P], _DT.float8e5, kind="ExternalInput").ap()
    sc_d = nc.dram_tensor("sc", [P, 3], _DT.float32, kind="ExternalInput").ap()
    bi_d = nc.dram_tensor("bi", [P, 3], _DT.float32, kind="ExternalInput").ap()
    out_d = nc.dram_tensor("out", [P, 1024], _DT.int16, kind="ExternalOutput").ap()

    shifts = [(_ALU.logical_shift_left, 5), (_ALU.logical_shift_left, 3),
              (_ALU.logical_shift_left, 1), (_ALU.logical_shift_right, 1)]

    with tile.TileContext(nc) as tc:
        with (
            tc.tile_pool(name="const", bufs=1) as cpool,
            tc.tile_pool(name="x", bufs=1) as xpool,
            tc.tile_pool(name="pl", bufs=6) as plpool,
            tc.tile_pool(name="psum", bufs=1, space="PSUM") as ppool,
            tc.tile_pool(name="dec", bufs=2) as dpool,
            tc.tile_pool(name="outp", bufs=1) as outpool,
        ):
            cab = xpool.tile([P, 8192], _DT.uint16)
            lhst = cpool.tile([P, 4 * 2 * P], _DT.float8e5)
            sc = cpool.tile([P, 3], _DT.float32)
            bi = cpool.tile([P, 3], _DT.float32)

            # single queue, strict order; [128,1024] chunks keep the HWDGE
            # fixed cost (625ns/DMA) under the transfer time (728ns)
            def chunk(i):
                nc.sync.dma_start(out=cab[:, i * 1024:(i + 1) * 1024],
                                  in_=cab_d[:, i * 1024:(i + 1) * 1024])
            chunk(0)
            chunk(1)
            nc.sync.dma_start(out=lhst[:], in_=lhst_d)
            chunk(2)
            chunk(3)
            chunk(4)
            chunk(5)
            chunk(6)
            chunk(7)
            nc.sync.dma_start(out=sc[:], in_=sc_d)
            nc.sync.dma_start(out=bi[:], in_=bi_d)

            # pair-3 rhs tiles: zero k-tile half written once by Pool,
            # g3 plane half rewritten per (hf, rh) by DVE
            p3 = [cpool.tile([P, 2048], _DT.uint16, name=f"p3_{i}")
                  for i in range(2)]
            for t in p3:
                nc.gpsimd.memset(t[:, :1024], 0.0)

            # PE p-state warmup: dummy DR matmuls on a zeroed scratch tile
            # during the DMA lead-in so PE hits full clock (2.4 GHz needs
            # 3us continuous busy) before the first real matmul
            scr = cpool.tile([P, 512], _DT.float8e5, name="scr")
            nc.vector.memset(scr[:], 0.0)
            wps = ppool.tile([P, 256], _DT.float32, name="warm", tag="warm")
            wlhs = scr[:].rearrange("p (t m) -> p t m", t=2)[:, :, :128]
            wrhs = scr[:].rearrange("p (t n) -> p t n", t=2)
            for _ in range(52):
                nc.tensor.matmul(wps[:], wlhs, wrhs,
                                 start=True, stop=True,
                                 perf_mode=mybir.MatmulPerfMode.DoubleRow)

            banks = [
                ppool.tile([P, 512], _DT.float32, name=f"bank{hf}", tag=f"bank{hf}")
                for hf in range(2)
            ]

            # planes + matmuls; the first block's pair passes split by src
            # half so DVE starts right after the first chunk lands
            def mms(pt, hf, rh, k):
                rhv = pt[:].bitcast(_DT.float8e5).rearrange(
                    "p (t r n) -> p t r n", t=2, r=4, n=512)
                ltr = lhst[:, k * 2 * P:(k + 1) * 2 * P].rearrange(
                    "p (t m) -> p t m", t=2)
                for r4 in range(4):
                    nc.tensor.matmul(
                        banks[hf][:],
                        ltr,
                        rhv[:, :, r4],
                        start=(rh == 0 and k == 0 and r4 == 0),
                        stop=(rh == 1 and k == 3 and r4 == 3),
                        perf_mode=mybir.MatmulPerfMode.DoubleRow,
                    )

            def p3_pass(hf, rh):
                base = hf * 4096 + rh * 2048
                pt = p3[(hf * 2 + rh) % 2]
                nc.vector.tensor_scalar(
                    out=pt[:, 1024:], in0=cab[:, base + 1024:base + 2048],
                    scalar1=1, scalar2=0x6060,
                    op0=_ALU.logical_shift_right, op1=_ALU.bitwise_and)
                mms(pt, hf, rh, 3)

            # hf0 blocks: 3 ca-half passes, then 3 cb-half passes + p3, so
            # DVE rides right behind the chunk stream
            for rh in range(2):
                base = rh * 2048
                pts0 = []
                for k in range(3):
                    pt = plpool.tile([P, 2048], _DT.uint16, name="pl", tag="pl")
                    op0, amt = shifts[k]
                    nc.vector.tensor_scalar(
                        out=pt[:, :1024], in0=cab[:, base:base + 1024],
                        scalar1=amt, scalar2=0x6060,
                        op0=op0, op1=_ALU.bitwise_and)
                    pts0.append(pt)
                for k in range(3):
                    pt = pts0[k]
                    op0, amt = shifts[k]
                    nc.vector.tensor_scalar(
                        out=pt[:, 1024:], in0=cab[:, base + 1024:base + 2048],
                        scalar1=amt, scalar2=0x6060,
                        op0=op0, op1=_ALU.bitwise_and)
                    mms(pt, 0, rh, k)
                p3_pass(0, rh)

            for rh in range(2):
                base = 4096 + rh * 2048
                for k in range(3):
                    pt = plpool.tile([P, 2048], _DT.uint16,
                                     name="pl", tag="pl")
                    op0, amt = shifts[k]
                    nc.vector.tensor_scalar(
                        out=pt[:], in0=cab[:, base:base + 2048],
                        scalar1=amt, scalar2=0x6060,
                        op0=op0, op1=_ALU.bitwise_and)
                    mms(pt, 1, rh, k)
                p3_pass(1, rh)

            # decode to per-slot maxima m [128, 512] per hf; the 8-slot fold
            # + index decode happen on the host during unshard.
            # hf0 (early bank): extraction/encode on ACT, overlapped with
            # hf1's planes/matmuls. hf1 (late, critical): all-DVE chain so
            # the tail has no cross-engine round-trips.
            def dtile(name, dt=_DT.int16):
                return dpool.tile([P, 512], dt, name=name, tag=name)

            out_t = outpool.tile([P, 1024], _DT.int16)
            S0, S1 = banks[0], banks[1]

            # ACT: trunc-extractions + e2 per hf (bank stops gate these);
            # DVE: bit-extracts, e1/e0 (4x TS), maxes
            t1a = dtile("t1a")
            nc.scalar.activation(t1a[:], S0[:], _ACTF.Identity,
                                 bias=0.0, scale=0.5)
            t9a = dtile("t9a")
            nc.scalar.activation(t9a[:], S0[:], _ACTF.Identity,
                                 bias=0.0, scale=1.0 / 512.0)
            e2a = dtile("e2a")
            nc.scalar.activation(e2a[:], t9a[:], _ACTF.Identity,
                                 bias=bi[:, 2:3], scale=sc[:, 2:3])
            t1b = dtile("t1b")
            nc.scalar.activation(t1b[:], S1[:], _ACTF.Identity,
                                 bias=0.0, scale=0.5)
            t9b = dtile("t9b")
            nc.scalar.activation(t9b[:], S1[:], _ACTF.Identity,
                                 bias=0.0, scale=1.0 / 512.0)
            e2b = dtile("e2b")
            nc.scalar.activation(e2b[:], t9b[:], _ACTF.Identity,
                                 bias=bi[:, 2:3], scale=sc[:, 2:3])

            # DVE hf0 chain (fills the planes-end -> bank1-stop gap)
            f1a = dtile("f1a")
            nc.vector.tensor_scalar(out=f1a[:], in0=t1a[:], scalar1=255,
                                    scalar2=None, op0=_ALU.bitwise_and)
            f0a = dtile("f0a", _DT.float16)
            nc.vector.scalar_tensor_tensor(
                out=f0a[:], in0=t1a[:], scalar=-2.0, in1=S0[:],
                op0=_ALU.mult, op1=_ALU.add)
            e1a = dtile("e1a")
            nc.vector.tensor_scalar(out=e1a[:], in0=f1a[:],
                                    scalar1=sc[:, 1:2], scalar2=bi[:, 1:2],
                                    op0=_ALU.mult, op1=_ALU.add)
            e0a = dtile("e0a")
            nc.vector.tensor_scalar(out=e0a[:], in0=f0a[:],
                                    scalar1=sc[:, 0:1], scalar2=bi[:, 0:1],
                                    op0=_ALU.mult, op1=_ALU.add)
            m01a = dtile("m01a")
            nc.vector.tensor_tensor(out=m01a[:], in0=e0a[:], in1=e1a[:],
                                    op=_ALU.max)
            nc.vector.tensor_tensor(out=out_t[:, :512],
                                    in0=m01a[:], in1=e2a[:], op=_ALU.max)
            nc.scalar.dma_start(out=out_d[:, :512], in_=out_t[:, :512])

            # DVE hf1 chain (terminal)
            f1b = dtile("f1b")
            nc.vector.tensor_scalar(out=f1b[:], in0=t1b[:], scalar1=255,
                                    scalar2=None, op0=_ALU.bitwise_and)
            f0b = dtile("f0b", _DT.float16)
            nc.vector.scalar_tensor_tensor(
                out=f0b[:], in0=t1b[:], scalar=-2.0, in1=S1[:],
                op0=_ALU.mult, op1=_ALU.add)
            e1b = dtile("e1b")
            nc.vector.tensor_scalar(out=e1b[:], in0=f1b[:],
                                    scalar1=sc[:, 1:2], scalar2=bi[:, 1:2],
                                    op0=_ALU.mult, op1=_ALU.add)
            e0b = dtile("e0b")
            nc.vector.tensor_scalar(out=e0b[:], in0=f0b[:],
                                    scalar1=sc[:, 0:1], scalar2=bi[:, 0:1],
                                    op0=_ALU.mult, op1=_ALU.add)
            m01b = dtile("m01b")
            nc.vector.tensor_tensor(out=m01b[:], in0=e0b[:], in1=e1b[:],
                                    op=_ALU.max)
            nc.vector.tensor_tensor(out=out_t[:, 512:],
                                    in0=m01b[:], in1=e2b[:], op=_ALU.max)
            nc.sync.dma_start(out=out_d[:, 512:], in_=out_t[:, 512:])
    nc.finalize()
    return nc


_CACHED = None


def _get_nc():
    global _CACHED
    if _CACHED is None:
        _CACHED = _build()
    return _CACHED


_LUTA, _LUTB = _luts()


def _pack(byte_a: np.ndarray, byte_b: np.ndarray) -> np.ndarray:
    """[512, 2048] u8 field bytes -> cab [128, 8192] u16.

    rows 512 = (pp 32, par 2, rh 2, r4 4); cols 2048 = (hf 2, wch 8, p 128)
    cab free = (hf 2, rh 2, src 2, r4 4, wch 8, pp 32), u16 = par lo/hi.
    """
    parts = []
    for X in (byte_a, byte_b):
        x = X.reshape(32, 2, 2, 4, 2, 8, 128)
        x = x.transpose(6, 4, 2, 3, 5, 0, 1)  # p, hf, rh, r4, wch, pp, par
        parts.append(x[..., 0].astype(np.uint16)
                     | (x[..., 1].astype(np.uint16) << 8))
    cab = np.stack(parts, axis=3)  # p, hf, rh, src, r4, wch, pp
    return np.ascontiguousarray(cab).reshape(P, 8192)


def kernel(labels: np.ndarray, class_weights: np.ndarray, dsf) -> np.ndarray:
    global LAST_RESULTS
    dsf = int(np.asarray(dsf))
    assert dsf == DSF, f"kernel hardcodes dsf=8, got {dsf}"
    labels = np.asarray(labels)
    out_dtype = labels.dtype

    lab = labels.reshape(B * H, W).astype(np.uint8)
    byte_a = _LUTA[lab]
    byte_b = _LUTB[lab]

    lhst_e5, sc, bi = _aux_arrays(class_weights)
    in_maps = []
    for k in range(NCORES):
        sl = slice(k * ROWS, (k + 1) * ROWS)
        in_maps.append({
            "cab": _pack(byte_a[sl], byte_b[sl]),
            "lhst": lhst_e5,
            "sc": sc,
            "bi": bi,
        })

    nc = _get_nc()
    res = run_bass_kernel_spmd(
        nc, in_maps, core_ids=list(range(NCORES)), trace=TRACE,
    )
    LAST_RESULTS = res

    # host unshard: fold 8 slots -> patch max M, decode c* = 32*u - M with
    # u = (M + 20) // 32  (E = 32*w*n - c + 2048 encode)
    modes = np.empty((B * GH, GW), dtype=np.int64)
    for k in range(NCORES):
        o = res.results[k]["out"]  # [128=(q16,g8), (hf2, wch8, pp32, par2)]
        M = o.reshape(16, 8, 1024).max(axis=1).astype(np.int32)  # [q, n]
        cstar = 32 * ((M + 20) >> 5) - M  # [16, (hf, wch, pp, par)]
        arr = cstar.reshape(16, 2, 8, 32, 2)  # q, hf, wch, pp, par
        blk = arr.transpose(3, 4, 1, 2, 0).reshape(PROWS, GW)
        modes[k * PROWS:(k + 1) * PROWS] = blk
    return modes.reshape(B, GH, GW).astype(out_dtype)


# revision 38
# speedup vs baseline: 1.0108x; 1.0108x over previous
"""ClassWeightedModalDownSampler Trainium2 kernel, v3.

Host packs each pixel's class c into two field-bytes interleaved in one
uint16 tensor cab [128, (hf 2, rh 2, src 2, r4 4, wch 8, pp 32)]:
  src=0 (ca): groups 0-2 as 2-bit fields t=c%3+1 at bits 2g..2g+1
  src=1 (cb): groups 4-6 at bits 2(g-4)..; group 3 at bits 6-7
(u16 packs two adjacent patch rows, lo/hi byte.)

Device: 4 uniform DVE tensor_scalar passes per (hf, rh) chunk over the
contiguous [ca-block | cb-block] columns:
  pass k<3: (x << (5-2k)) & 0x6060 -> [plane k | plane k+4]
  pass 3:   (x >> 1)      & 0x6060 -> [zeros   | plane 3  ]
Each output [128, 2048] u16 is bitcast to fp8e5 ({0,32,64,96} bytes =
{0, 2^-7, 2, 2^9}) and consumed by DoubleRow matmuls (2 k-tiles/pair)
with a block one-hot lhsT mapping partition p, pair k, tile t to PSUM
slot m = 8*(p//8) + group: S[8q+g, n] = n0*2^-7 + n1*2 + n2*512 exactly.

Decode per hf bank (all int16, A=32 encode E=32*w*n - c):
  t1 = trunc(S/2) (ACT) ; t9 = t1>>8 (DVE 4x) ; f1 = t1&255 (DVE 4x)
  f0 = S - 2*t1 (Pool STT, fp16) ; e_d = sc_d*f_d + bi_d (ACT/DVE-TS)
  m = max(e0,e1,e2) (DVE/Pool) ; PE-transpose m -> slots in free dim ;
  R = reduce_max over 8-slot groups (DVE) ; c* = 32*((R+20)>>5) - R.
Output [128, (hf, j, q)] int16, unscrambled on host. Exact for integer
class_weights with 32*64*max(w) < 32768 (reference w in {1,10}).
"""

import numpy as np
import ml_dtypes

import concourse.bass as bass
import concourse.mybir as mybir
import concourse.tile as tile
from concourse import bacc
from concourse.bass_utils import run_bass_kernel_spmd

NCORES = 8
B, H, W = 4, 1024, 2048
DSF = 8
NCLS = 20
GH, GW = H // DSF, W // DSF
ROWS = (B * H) // NCORES     # 512 label rows per core
PROWS = ROWS // DSF          # 64 patch rows per core
P = 128

_DT = mybir.dt
_ALU = mybir.AluOpType
_ACTF = mybir.ActivationFunctionType

TRACE = False
LAST_RESULTS = None

# encode offset: E = 32*w*n - c + 2048 keeps every value in [1024, 31744)
# whose int16 bits are a NORMAL fp16 pattern (exact through PE transpose)
EOFF = 2048.0
NEGB = 1024.0  # pad slots: below any real slot's E


def _luts():
    """Per-class field bytes: (ca, cb)."""
    la = np.zeros(256, dtype=np.uint8)
    lb = np.zeros(256, dtype=np.uint8)
    for c in range(NCLS):
        g, d = divmod(c, 3)
        t = d + 1
        if g < 3:
            la[c] = t << (2 * g)
        elif g == 3:
            lb[c] = t << 6
        else:
            lb[c] = t << (2 * (g - 4))
    return la, lb


def _aux_arrays(class_weights: np.ndarray):
    w = np.asarray(class_weights, dtype=np.float32)
    assert w.shape[0] == NCLS
    # lhsT [p, pair 4, t 2, m 128] one-hot: m = 8*(p//8) + group(pair, t)
    # group(k,0)=k (k<3), group(3,0)=None (zero half), group(k,1)=k+4 (k<3),
    # group(3,1)=3.
    lhst = np.zeros((P, 4, 2, P), dtype=np.float32)
    for p in range(P):
        mb = 8 * (p // 8)
        for k in range(4):
            for t in range(2):
                if k < 3:
                    g = k if t == 0 else k + 4
                elif t == 1:
                    g = 3
                else:
                    continue
                lhst[p, k, t, mb + g] = 1.0
    lhst_e5 = lhst.reshape(P, 4 * 2 * P).astype(ml_dtypes.float8_e5m2)

    # encode scale/bias [128, 3] (col d); partition m -> g = m % 8
    sc = np.zeros((P, 3), dtype=np.float32)
    bi = np.zeros((P, 3), dtype=np.float32)
    for m in range(P):
        g = m % 8
        for d in range(3):
            c = 3 * g + d
            if g == 7 or c >= NCLS:
                sc[m, d] = 0.0
                bi[m, d] = NEGB
            else:
                sc[m, d] = 32.0 * w[c] * (128.0 if d == 0 else 1.0)
                bi[m, d] = EOFF - float(c)
    return lhst_e5, sc, bi


def _build():
    nc = bacc.Bacc(
        "TRN2",
        target_bir_lowering=False,
        debug=False,
        num_devices=NCORES,
    )
    cab_d = nc.dram_tensor("cab", [P, 8192], _DT.uint16, kind="ExternalInput").ap()
    lhst_d = nc.dram_tensor("lhst", [P, 4 * 2 * P], _DT.float8e5, kind="ExternalInput").ap()
    sc_d = nc.dram_tensor("sc", [P, 3], _DT.float32, kind="ExternalInput").ap()
    bi_d = nc.dram_tensor("bi", [P, 3], _DT.float32, kind="ExternalInput").ap()
    out_d = nc.dram_tensor("out", [P, 1024], _DT.int16, kind="ExternalOutput").ap()

    shifts = [(_ALU.logical_shift_left, 5), (_ALU.logical_shift_left, 3),
              (_ALU.logical_shift_left, 1), (_ALU.logical_shift_right, 1)]

    with tile.TileContext(nc) as tc:
        with (
            tc.tile_pool(name="const", bufs=1) as cpool,
            tc.tile_pool(name="x", bufs=1) as xpool,
            tc.tile_pool(name="pl", bufs=6) as plpool,
            tc.tile_pool(name="psum", bufs=1, space="PSUM") as ppool,
            tc.tile_pool(name="dec", bufs=2) as dpool,
            tc.tile_pool(name="outp", bufs=1) as outpool,
        ):
            cab = xpool.tile([P, 8192], _DT.uint16)
            lhst = cpool.tile([P, 4 * 2 * P], _DT.float8e5)
            sc = cpool.tile([P, 3], _DT.float32)
            bi = cpool.tile([P, 3], _DT.float32)

            # single queue, strict order; [128,1024] chunks keep the HWDGE
            # fixed cost (625ns/DMA) under the transfer time (728ns)
            def chunk(i):
                nc.sync.dma_start(out=cab[:, i * 1024:(i + 1) * 1024],
                                  in_=cab_d[:, i * 1024:(i + 1) * 1024])
            chunk(0)
            chunk(1)
            nc.sync.dma_start(out=lhst[:], in_=lhst_d)
            chunk(2)
            chunk(3)
            chunk(4)
            chunk(5)
            chunk(6)
            chunk(7)
            nc.sync.dma_start(out=sc[:], in_=sc_d)
            nc.sync.dma_start(out=bi[:], in_=bi_d)

            # pair-3 rhs tiles: zero k-tile half written once by Pool,
            # g3 plane half rewritten per (hf, rh) by DVE
            p3 = [cpool.tile([P, 2048], _DT.uint16, name=f"p3_{i}")
                  for i in range(2)]
            for t in p3:
                nc.gpsimd.memset(t[:, :1024], 0.0)

            # PE p-state warmup: dummy DR matmuls on a zeroed scratch tile
            # during the DMA lead-in so PE hits full clock (2.4 GHz needs
            # 3us continuous busy) before the first real matmul
            scr = cpool.tile([P, 512], _DT.float8e5, name="scr")
            nc.vector.memset(scr[:], 0.0)
            wps = ppool.tile([P, 256], _DT.float32, name="warm", tag="warm")
            wlhs = scr[:].rearrange("p (t m) -> p t m", t=2)[:, :, :128]
            wrhs = scr[:].rearrange("p (t n) -> p t n", t=2)
            for _ in range(52):
                nc.tensor.matmul(wps[:], wlhs, wrhs,
                                 start=True, stop=True,
                                 perf_mode=mybir.MatmulPerfMode.DoubleRow)

            banks = [
                ppool.tile([P, 512], _DT.float32, name=f"bank{hf}", tag=f"bank{hf}")
                for hf in range(2)
            ]

            # planes + matmuls; the first block's pair passes split by src
            # half so DVE starts right after the first chunk lands
            def mms(pt, hf, rh, k):
                rhv = pt[:].bitcast(_DT.float8e5).rearrange(
                    "p (t r n) -> p t r n", t=2, r=4, n=512)
                ltr = lhst[:, k * 2 * P:(k + 1) * 2 * P].rearrange(
                    "p (t m) -> p t m", t=2)
                for r4 in range(4):
                    nc.tensor.matmul(
                        banks[hf][:],
                        ltr,
                        rhv[:, :, r4],
                        start=(rh == 0 and k == 0 and r4 == 0),
                        stop=(rh == 1 and k == 3 and r4 == 3),
                        perf_mode=mybir.MatmulPerfMode.DoubleRow,
                    )

            def p3_pass(hf, rh):
                base = hf * 4096 + rh * 2048
                pt = p3[(hf * 2 + rh) % 2]
                nc.vector.tensor_scalar(
                    out=pt[:, 1024:], in0=cab[:, base + 1024:base + 2048],
                    scalar1=1, scalar2=0x6060,
                    op0=_ALU.logical_shift_right, op1=_ALU.bitwise_and)
                mms(pt, hf, rh, 3)

            # hf0 blocks: 3 ca-half passes, then 3 cb-half passes + p3, so
            # DVE rides right behind the chunk stream
            for rh in range(2):
                base = rh * 2048
                pts0 = []
                for k in range(3):
                    pt = plpool.tile([P, 2048], _DT.uint16, name="pl", tag="pl")
                    op0, amt = shifts[k]
                    nc.vector.tensor_scalar(
                        out=pt[:, :1024], in0=cab[:, base:base + 1024],
                        scalar1=amt, scalar2=0x6060,
                        op0=op0, op1=_ALU.bitwise_and)
                    pts0.append(pt)
                for k in range(3):
                    pt = pts0[k]
                    op0, amt = shifts[k]
                    nc.vector.tensor_scalar(
                        out=pt[:, 1024:], in0=cab[:, base + 1024:base + 2048],
                        scalar1=amt, scalar2=0x6060,
                        op0=op0, op1=_ALU.bitwise_and)
                    mms(pt, 0, rh, k)
                p3_pass(0, rh)

            for rh in range(2):
                base = 4096 + rh * 2048
                for k in range(3):
                    pt = plpool.tile([P, 2048], _DT.uint16,
                                     name="pl", tag="pl")
                    op0, amt = shifts[k]
                    nc.vector.tensor_scalar(
                        out=pt[:], in0=cab[:, base:base + 2048],
                        scalar1=amt, scalar2=0x6060,
                        op0=op0, op1=_ALU.bitwise_and)
                    mms(pt, 1, rh, k)
                p3_pass(1, rh)

            # decode to per-slot maxima m [128, 512] per hf; the 8-slot fold
            # + index decode happen on the host during unshard.
            # hf0 (early bank): extraction/encode on ACT, overlapped with
            # hf1's planes/matmuls. hf1 (late, critical): all-DVE chain so
            # the tail has no cross-engine round-trips.
            def dtile(name, dt=_DT.int16):
                return dpool.tile([P, 512], dt, name=name, tag=name)

            out_t = outpool.tile([P, 1024], _DT.int16)
            S0, S1 = banks[0], banks[1]

            # ACT: trunc-extractions + e2 per hf (bank stops gate these);
            # DVE: bit-extracts, e1/e0 (4x TS), maxes
            t1a = dtile("t1a")
            nc.scalar.activation(t1a[:], S0[:], _ACTF.Identity,
                                 bias=0.0, scale=0.5)
            t9a = dtile("t9a")
            nc.scalar.activation(t9a[:], S0[:], _ACTF.Identity,
                                 bias=0.0, scale=1.0 / 512.0)
            e2a = dtile("e2a")
            nc.scalar.activation(e2a[:], t9a[:], _ACTF.Identity,
                                 bias=bi[:, 2:3], scale=sc[:, 2:3])
            t1b = dtile("t1b")
            nc.scalar.activation(t1b[:], S1[:], _ACTF.Identity,
                                 bias=0.0, scale=0.5)
            t9b = dtile("t9b")
            nc.scalar.activation(t9b[:], S1[:], _ACTF.Identity,
                                 bias=0.0, scale=1.0 / 512.0)
            e2b = dtile("e2b")
            nc.scalar.activation(e2b[:], t9b[:], _ACTF.Identity,
                                 bias=bi[:, 2:3], scale=sc[:, 2:3])

            # DVE hf0 chain (fills the planes-end -> bank1-stop gap)
            f1a = dtile("f1a")
            nc.vector.tensor_scalar(out=f1a[:], in0=t1a[:], scalar1=255,
                                    scalar2=None, op0=_ALU.bitwise_and)
            f0a = dtile("f0a", _DT.float16)
            nc.vector.scalar_tensor_tensor(
                out=f0a[:], in0=t1a[:], scalar=-2.0, in1=S0[:],
                op0=_ALU.mult, op1=_ALU.add)
            e1a = dtile("e1a")
            nc.vector.tensor_scalar(out=e1a[:], in0=f1a[:],
                                    scalar1=sc[:, 1:2], scalar2=bi[:, 1:2],
                                    op0=_ALU.mult, op1=_ALU.add)
            e0a = dtile("e0a")
            nc.vector.tensor_scalar(out=e0a[:], in0=f0a[:],
                                    scalar1=sc[:, 0:1], scalar2=bi[:, 0:1],
                                    op0=_ALU.mult, op1=_ALU.add)
            m01a = dtile("m01a")
            nc.vector.tensor_tensor(out=m01a[:], in0=e1a[:], in1=e2a[:],
                                    op=_ALU.max)
            nc.vector.tensor_tensor(out=out_t[:, :512],
                                    in0=m01a[:], in1=e0a[:], op=_ALU.max)
            nc.scalar.dma_start(out=out_d[:, :512], in_=out_t[:, :512])

            # DVE hf1 chain (terminal)
            f1b = dtile("f1b")
            nc.vector.tensor_scalar(out=f1b[:], in0=t1b[:], scalar1=255,
                                    scalar2=None, op0=_ALU.bitwise_and)
            f0b = dtile("f0b", _DT.float16)
            nc.vector.scalar_tensor_tensor(
                out=f0b[:], in0=t1b[:], scalar=-2.0, in1=S1[:],
                op0=_ALU.mult, op1=_ALU.add)
            e1b = dtile("e1b")
            nc.vector.tensor_scalar(out=e1b[:], in0=f1b[:],
                                    scalar1=sc[:, 1:2], scalar2=bi[:, 1:2],
                                    op0=_ALU.mult, op1=_ALU.add)
            e0b = dtile("e0b")
            nc.vector.tensor_scalar(out=e0b[:], in0=f0b[:],
                                    scalar1=sc[:, 0:1], scalar2=bi[:, 0:1],
                                    op0=_ALU.mult, op1=_ALU.add)
            m01b = dtile("m01b")
            nc.vector.tensor_tensor(out=m01b[:], in0=e1b[:], in1=e2b[:],
                                    op=_ALU.max)
            nc.vector.tensor_tensor(out=out_t[:, 512:],
                                    in0=m01b[:], in1=e0b[:], op=_ALU.max)
            nc.sync.dma_start(out=out_d[:, 512:], in_=out_t[:, 512:])
    nc.finalize()
    return nc


_CACHED = None


def _get_nc():
    global _CACHED
    if _CACHED is None:
        _CACHED = _build()
    return _CACHED


_LUTA, _LUTB = _luts()


def _pack(byte_a: np.ndarray, byte_b: np.ndarray) -> np.ndarray:
    """[512, 2048] u8 field bytes -> cab [128, 8192] u16.

    rows 512 = (pp 32, par 2, rh 2, r4 4); cols 2048 = (hf 2, wch 8, p 128)
    cab free = (hf 2, rh 2, src 2, r4 4, wch 8, pp 32), u16 = par lo/hi.
    """
    parts = []
    for X in (byte_a, byte_b):
        x = X.reshape(32, 2, 2, 4, 2, 8, 128)
        x = x.transpose(6, 4, 2, 3, 5, 0, 1)  # p, hf, rh, r4, wch, pp, par
        parts.append(x[..., 0].astype(np.uint16)
                     | (x[..., 1].astype(np.uint16) << 8))
    cab = np.stack(parts, axis=3)  # p, hf, rh, src, r4, wch, pp
    return np.ascontiguousarray(cab).reshape(P, 8192)


def kernel(labels: np.ndarray, class_weights: np.ndarray, dsf) -> np.ndarray:
    global LAST_RESULTS
    dsf = int(np.asarray(dsf))
    assert dsf == DSF, f"kernel hardcodes dsf=8, got {dsf}"
    labels = np.asarray(labels)
    out_dtype = labels.dtype

    lab = labels.reshape(B * H, W).astype(np.uint8)
    byte_a = _LUTA[lab]
    byte_b = _LUTB[lab]

    lhst_e5, sc, bi = _aux_arrays(class_weights)
    in_maps = []
    for k in range(NCORES):
        sl = slice(k * ROWS, (k + 1) * ROWS)
        in_maps.append({
            "cab": _pack(byte_a[sl], byte_b[sl]),
            "lhst": lhst_e5,
            "sc": sc,
            "bi": bi,
        })

    nc = _get_nc()
    res = run_bass_kernel_spmd(
        nc, in_maps, core_ids=list(range(NCORES)), trace=TRACE,
    )
    LAST_RESULTS = res

    # host unshard: fold 8 slots -> patch max M, decode c* = 32*u - M with
    # u = (M + 20) // 32  (E = 32*w*n - c + 2048 encode)
    modes = np.empty((B * GH, GW), dtype=np.int64)
    for k in range(NCORES):
        o = res.results[k]["out"]  # [128=(q16,g8), (hf2, wch8, pp32, par2)]
        M = o.reshape(16, 8, 1024).max(axis=1).astype(np.int32)  # [q, n]
        cstar = 32 * ((M + 20) >> 5) - M  # [16, (hf, wch, pp, par)]
        arr = cstar.reshape(16, 2, 8, 32, 2)  # q, hf, wch, pp, par
        blk = arr.transpose(3, 4, 1, 2, 0).reshape(PROWS, GW)
        modes[k * PROWS:(k + 1) * PROWS] = blk
    return modes.reshape(B, GH, GW).astype(out_dtype)
